# revision 7
# baseline (speedup 1.0000x reference)
"""Trainium2 Bass kernel for IntraTemporalAttention.

Data-parallel over batch: 16 batches -> 8 NeuronCores, 2 batches/core.
Each core streams its [4096, 2, 512] shards of encoder_features /
encoder_out in 32 chunks of 128 seq positions, laid out [128s, (b,h)]
in SBUF (4KB contiguous DMA lines).

Per chunk:
  DVE: x = ef + dec_bcast                     [128, 1024]
  ACT: x = tanh(x)                            [128, 1024]
  DVE: scalar_tensor_tensor (x * v) with fused h-sum -> scores col (x2, per b)
  ACT: exp on the 2 score cols
  DVE: att1 = exp * (mask/ts) cols
  PE : ctx_psum[b] += att1_col.T @ enc_chunk_b   (accumulated over 32 chunks)

Normalization by sum_s(att1) commutes with the matvec, so it is applied
once at the end to both the attention output and the context.
"""

from contextlib import ExitStack

import numpy as np

import concourse.bacc as bacc
import concourse.mybir as mybir
import concourse.tile as tile
from concourse import masks
from concourse.bass_utils import run_bass_kernel_spmd

SEQ, BATCH, HIDDEN = 4096, 16, 512
N_CORES = 8
BL = BATCH // N_CORES  # local batches per core = 2
P = 128                # SBUF partitions
NCHUNK = SEQ // P      # 32
KC = HIDDEN // P       # 4 k-chunks of W
FDT = mybir.dt.float32
F32 = mybir.AluOpType


def build_nc():
    nc = bacc.Bacc("TRN2", debug=False, target_bir_lowering=False)

    oh = nc.declare_dram_parameter("outputs_hidden", [BL, HIDDEN], FDT, isOutput=False)
    enc = nc.declare_dram_parameter("encoder_out", [SEQ, BL, HIDDEN], FDT, isOutput=False)
    ef = nc.declare_dram_parameter("encoder_features", [SEQ, BL, HIDDEN], FDT, isOutput=False)
    msk = nc.declare_dram_parameter("encoder_mask", [BL, SEQ], FDT, isOutput=False)
    tss = nc.declare_dram_parameter("temporal_scores_sum", [BL, SEQ], FDT, isOutput=False)
    wf = nc.declare_dram_parameter("W_feat", [HIDDEN, HIDDEN], FDT, isOutput=False)
    bf = nc.declare_dram_parameter("b_feat", [HIDDEN], FDT, isOutput=False)
    va = nc.declare_dram_parameter("v_attn", [HIDDEN], FDT, isOutput=False)

    ctx_o = nc.declare_dram_parameter("context", [BL, HIDDEN], FDT, isOutput=True)
    att_o = nc.declare_dram_parameter("attention", [BL, SEQ], FDT, isOutput=True)
    nss_o = nc.declare_dram_parameter("new_scores_sum", [BL, SEQ], FDT, isOutput=True)

    with tile.TileContext(nc) as tc, ExitStack() as ctx:
        const = ctx.enter_context(tc.tile_pool(name="const", bufs=1))
        work = ctx.enter_context(tc.tile_pool(name="work", bufs=3))
        prodp = ctx.enter_context(tc.tile_pool(name="prod", bufs=4))
        psum_acc = ctx.enter_context(tc.tile_pool(name="psum_acc", bufs=1, space="PSUM"))
        psum_scr = ctx.enter_context(tc.tile_pool(name="psum_scr", bufs=2, space="PSUM"))

        # ---------- constants ----------
        ones_row = const.tile([1, P], FDT, tag="ones_row")
        nc.vector.memset(ones_row[:], 1.0)
        ones_col = const.tile([P, 1], FDT, tag="ones_col")
        nc.vector.memset(ones_col[:], 1.0)
        ident = const.tile([P, P], FDT, tag="ident")
        masks.make_identity(nc, ident[:])

        # ---------- preamble: dec[b,k] = sum_h oh[b,h] W[k,h] + bf[k] ----------
        oh_rows = []
        for b in range(BL):
            t = const.tile([1, HIDDEN], FDT, tag=f"oh_row{b}")
            nc.sync.dma_start(t[:], oh.ap()[b : b + 1, :])
            oh_rows.append(t)

        w_sb = const.tile([P, KC * HIDDEN], FDT, tag="w_sb")  # col = kc*512 + h
        nc.sync.dma_start(w_sb[:], wf.ap().rearrange("(kc p) h -> p kc h", p=P))

        bf_sb = const.tile([P, KC], FDT, tag="bf_sb")  # [p, kc] = bf[kc*128+p]
        nc.sync.dma_start(bf_sb[:], bf.ap().rearrange("(kc p) -> p kc", p=P))

        v_row = const.tile([1, HIDDEN], FDT, tag="v_row")
        nc.sync.dma_start(v_row[:], va.ap().rearrange("(o h) -> o h", o=1))

        # broadcast oh rows across 128 partitions (outer product with ones)
        oh_bc = const.tile([P, BL * HIDDEN], FDT, tag="oh_bc")  # col = b*512 + h
        for b in range(BL):
            pscr = psum_scr.tile([P, HIDDEN], FDT, tag="pscr")
            nc.tensor.matmul(pscr[:], ones_row[:], oh_rows[b][:], start=True, stop=True)
            nc.scalar.copy(oh_bc[:, b * HIDDEN : (b + 1) * HIDDEN], pscr[:])

        v_bc = const.tile([P, HIDDEN], FDT, tag="v_bc")
        pscr = psum_scr.tile([P, HIDDEN], FDT, tag="pscr")
        nc.tensor.matmul(pscr[:], ones_row[:], v_row[:], start=True, stop=True)
        nc.scalar.copy(v_bc[:], pscr[:])

        # decT[p=k_lo, b*KC+kc] = dec[b, kc*128+p]
        decT = const.tile([P, BL * KC], FDT, tag="decT")
        for b in range(BL):
            for kc in range(KC):
                scr = prodp.tile([P, HIDDEN], FDT, tag="scr")
                c = b * KC + kc
                nc.vector.scalar_tensor_tensor(
                    out=scr[:],
                    in0=w_sb[:, kc * HIDDEN : (kc + 1) * HIDDEN],
                    scalar=1.0,
                    in1=oh_bc[:, b * HIDDEN : (b + 1) * HIDDEN],
                    op0=F32.mult,
                    op1=F32.mult,
                    accum_out=decT[:, c : c + 1],
                )
        for b in range(BL):
            nc.vector.tensor_add(
                decT[:, b * KC : (b + 1) * KC], decT[:, b * KC : (b + 1) * KC], bf_sb[:]
            )

        # transpose [128, BL*KC] -> [BL*KC, 128], row r=(b,kc) holds dec[b, kc*128 : ...]
        pdt = psum_scr.tile([BL * KC, P], FDT, tag="pscr", name="pdt")
        nc.tensor.transpose(pdt[:], decT[:], ident[:])
        dec8 = const.tile([BL * KC, P], FDT, tag="dec8")
        nc.scalar.copy(dec8[:], pdt[:])

        # gather rows into a single partition: dec_row[0, b*512 + kc*128 + j]
        dec_row = const.tile([1, BL * HIDDEN], FDT, tag="dec_row")
        nc.sync.dma_start(dec_row[:].rearrange("o (r j) -> o r j", r=BL * KC), dec8[:])

        # broadcast dec_row across partitions
        dec_bc = const.tile([P, BL * HIDDEN], FDT, tag="dec_bc")
        for half in range(BL):
            pscr = psum_scr.tile([P, HIDDEN], FDT, tag="pscr")
            nc.tensor.matmul(
                pscr[:], ones_row[:], dec_row[:, half * HIDDEN : (half + 1) * HIDDEN],
                start=True, stop=True,
            )
            nc.scalar.copy(dec_bc[:, half * HIDDEN : (half + 1) * HIDDEN], pscr[:])

        # ---------- mask / temporal_scores_sum, laid out [p, (c, b)] ----------
        # col = c*BL + b ; element (p, c, b) <-> DRAM (b, c*128+p)
        ts_all = const.tile([P, NCHUNK * BL], FDT, tag="ts_all")
        mk_all = const.tile([P, NCHUNK * BL], FDT, tag="mk_all")
        for b in range(BL):
            bcols = ts_all[:].rearrange("p (c b) -> p c b", b=BL)[:, :, b : b + 1]
            nc.sync.dma_start(bcols, tss.ap()[b].rearrange("(c p) -> p c", p=P))
            bcols = mk_all[:].rearrange("p (c b) -> p c b", b=BL)[:, :, b : b + 1]
            nc.sync.dma_start(bcols, msk.ap()[b].rearrange("(c p) -> p c", p=P))
        minv = const.tile([P, NCHUNK * BL], FDT, tag="minv")
        nc.vector.reciprocal(minv[:], ts_all[:])
        nc.vector.tensor_mul(minv[:], minv[:], mk_all[:])

        scores = const.tile([P, NCHUNK * BL], FDT, tag="scores")
        exp_all = const.tile([P, NCHUNK * BL], FDT, tag="exp_all")
        att1 = const.tile([P, NCHUNK * BL], FDT, tag="att1")

        pctx = [
            psum_acc.tile([1, HIDDEN], FDT, tag=f"pctx{b}", name=f"pctx{b}")
            for b in range(BL)
        ]

        # ---------- main streaming loop ----------
        for c in range(NCHUNK):
            eft = work.tile([P, BL * HIDDEN], FDT, tag="eft")
            nc.sync.dma_start(eft[:], ef.ap()[c * P : (c + 1) * P])
            enct = work.tile([P, BL * HIDDEN], FDT, tag="enct")
            nc.sync.dma_start(enct[:], enc.ap()[c * P : (c + 1) * P])

            nc.vector.tensor_add(eft[:], eft[:], dec_bc[:])
            nc.scalar.activation(eft[:], eft[:], mybir.ActivationFunctionType.Tanh)

            for b in range(BL):
                col = c * BL + b
                scr = prodp.tile([P, HIDDEN], FDT, tag="scr")
                nc.vector.scalar_tensor_tensor(
                    out=scr[:],
                    in0=eft[:, b * HIDDEN : (b + 1) * HIDDEN],
                    scalar=1.0,
                    in1=v_bc[:],
                    op0=F32.mult,
                    op1=F32.mult,
                    accum_out=scores[:, col : col + 1],
                )

            cols = slice(c * BL, (c + 1) * BL)
            nc.scalar.activation(
                exp_all[:, cols], scores[:, cols], mybir.ActivationFunctionType.Exp
            )
            nc.vector.tensor_mul(att1[:, cols], exp_all[:, cols], minv[:, cols])

            for b in range(BL):
                col = c * BL + b
                nc.tensor.matmul(
                    pctx[b][:],
                    att1[:, col : col + 1],
                    enct[:, b * HIDDEN : (b + 1) * HIDDEN],
                    start=(c == 0),
                    stop=(c == NCHUNK - 1),
                )

        # ---------- epilogue ----------
        # denom[b] = sum over (p, c) of att1
        part = const.tile([P, BL], FDT, tag="part")
        nc.vector.reduce_sum(
            part[:], att1[:].rearrange("p (c b) -> p b c", b=BL), axis=mybir.AxisListType.X
        )
        pden = psum_scr.tile([1, BL], FDT, tag="pscr", name="pden")
        nc.tensor.matmul(pden[:], ones_col[:], part[:], start=True, stop=True)
        rd_row = const.tile([1, BL], FDT, tag="rd_row")
        nc.vector.reciprocal(rd_row[:], pden[:])

        prd = psum_scr.tile([P, BL], FDT, tag="pscr", name="prd")
        nc.tensor.matmul(prd[:], ones_row[:], rd_row[:], start=True, stop=True)
        rd_bc = const.tile([P, BL], FDT, tag="rd_bc")
        nc.scalar.copy(rd_bc[:], prd[:])

        att_f = const.tile([P, NCHUNK * BL], FDT, tag="att_f")
        for b in range(BL):
            nc.vector.tensor_scalar_mul(
                att_f[:].rearrange("p (c b) -> p c b", b=BL)[:, :, b : b + 1],
                att1[:].rearrange("p (c b) -> p c b", b=BL)[:, :, b : b + 1],
                rd_bc[:, b : b + 1],
            )

        nss = const.tile([P, NCHUNK * BL], FDT, tag="nss")
        nc.vector.tensor_add(nss[:], exp_all[:], ts_all[:])

        for b in range(BL):
            nc.sync.dma_start(
                att_o.ap()[b].rearrange("(c p) -> p c", p=P),
                att_f[:].rearrange("p (c b) -> p c b", b=BL)[:, :, b : b + 1],
            )
            nc.sync.dma_start(
                nss_o.ap()[b].rearrange("(c p) -> p c", p=P),
                nss[:].rearrange("p (c b) -> p c b", b=BL)[:, :, b : b + 1],
            )

        ctx_sb = const.tile([1, BL * HIDDEN], FDT, tag="ctx_sb")
        for b in range(BL):
            nc.vector.tensor_scalar_mul(
                ctx_sb[:, b * HIDDEN : (b + 1) * HIDDEN], pctx[b][:], rd_row[:, b : b + 1]
            )
        nc.sync.dma_start(ctx_o.ap(), ctx_sb[:])

    nc.compile()
    return nc


_NC = None


def _get_nc():
    global _NC
    if _NC is None:
        _NC = build_nc()
    return _NC


def _shard(inputs, i):
    b0, b1 = i * BL, (i + 1) * BL
    f32 = lambda a: np.ascontiguousarray(np.asarray(a), dtype=np.float32)
    return {
        "outputs_hidden": f32(inputs["outputs_hidden"][0, b0:b1, :]),
        "encoder_out": f32(inputs["encoder_out"][:, b0:b1, :]),
        "encoder_features": f32(inputs["encoder_features"][:, b0:b1, :]),
        "encoder_mask": f32(inputs["encoder_mask"][b0:b1, 0, :]),
        "temporal_scores_sum": f32(inputs["temporal_scores_sum"][b0:b1, 0, :]),
        "W_feat": f32(inputs["W_feat"]),
        "b_feat": f32(inputs["b_feat"]),
        "v_attn": f32(inputs["v_attn"]),
    }


def kernel(**inputs):
    nc = _get_nc()
    in_maps = [_shard(inputs, i) for i in range(N_CORES)]
    res = run_bass_kernel_spmd(nc, in_maps, list(range(N_CORES))).results

    context = np.empty((1, BATCH, HIDDEN), np.float32)
    attention = np.empty((BATCH, 1, SEQ), np.float32)
    new_scores_sum = np.empty((BATCH, 1, SEQ), np.float32)
    for i in range(N_CORES):
        b0, b1 = i * BL, (i + 1) * BL
        context[0, b0:b1] = res[i]["context"].reshape(BL, HIDDEN)
        attention[b0:b1, 0] = res[i]["attention"]
        new_scores_sum[b0:b1, 0] = res[i]["new_scores_sum"]
    return context, attention, new_scores_sum


# revision 9
# speedup vs baseline: 1.1254x; 1.1254x over previous
"""Trainium2 Bass kernel for IntraTemporalAttention.

Data-parallel over batch: 16 batches -> 8 NeuronCores, 2 batches/core.
Each core streams its [4096, 2, 512] shards of encoder_features /
encoder_out in 32 chunks of 128 seq positions, laid out [128s, (b,h)]
in SBUF (4KB contiguous DMA lines).

Per chunk:
  POOL/DVE: x = ef + dec_bcast (one batch-half per engine)  [128, 1024]
  ACT:  x = tanh(x)                                         [128, 1024]
  DVE:  scalar_tensor_tensor (x * v) with fused h-sum -> scores col (x2)
  ACT:  exp on the 2 score cols
  DVE:  att1 = exp * (mask/ts) cols
  PE :  ctx_psum[b] += att1_col.T @ enc_chunk_b  (fp32r, accumulated)

Normalization by sum_s(att1) commutes with the matvec, so it is applied
once at the end to both the attention output and the context.

Small [b, 4096] tensors (mask, temporal_scores_sum in; attention,
new_scores_sum out) are staged [32c, 128p]-contiguous in SBUF and
PE-transposed to/from the compute layout - a straight strided DMA
would shatter into 4-byte packets and choke the DMA engines.
"""

from contextlib import ExitStack

import numpy as np

import concourse.bacc as bacc
import concourse.mybir as mybir
import concourse.tile as tile
from concourse import masks
from concourse.bass_utils import run_bass_kernel_spmd

SEQ, BATCH, HIDDEN = 4096, 16, 512
N_CORES = 8
BL = BATCH // N_CORES  # local batches per core = 2
P = 128                # SBUF partitions
NCHUNK = SEQ // P      # 32
KC = HIDDEN // P       # 4 k-chunks of W
FDT = mybir.dt.float32
RDT = mybir.dt.float32r
F32 = mybir.AluOpType
AFT = mybir.ActivationFunctionType


def build_nc():
    nc = bacc.Bacc("TRN2", debug=False, target_bir_lowering=False)

    oh = nc.declare_dram_parameter("outputs_hidden", [BL, HIDDEN], FDT, isOutput=False)
    enc = nc.declare_dram_parameter("encoder_out", [SEQ, BL, HIDDEN], FDT, isOutput=False)
    ef = nc.declare_dram_parameter("encoder_features", [SEQ, BL, HIDDEN], FDT, isOutput=False)
    msk = nc.declare_dram_parameter("encoder_mask", [BL, SEQ], FDT, isOutput=False)
    tss = nc.declare_dram_parameter("temporal_scores_sum", [BL, SEQ], FDT, isOutput=False)
    wf = nc.declare_dram_parameter("W_feat", [HIDDEN, HIDDEN], FDT, isOutput=False)
    bf = nc.declare_dram_parameter("b_feat", [HIDDEN], FDT, isOutput=False)
    va = nc.declare_dram_parameter("v_attn", [HIDDEN], FDT, isOutput=False)

    ctx_o = nc.declare_dram_parameter("context", [BL, HIDDEN], FDT, isOutput=True)
    att_o = nc.declare_dram_parameter("attention", [BL, SEQ], FDT, isOutput=True)
    nss_o = nc.declare_dram_parameter("new_scores_sum", [BL, SEQ], FDT, isOutput=True)

    with tile.TileContext(nc) as tc, ExitStack() as ctx:
        const = ctx.enter_context(tc.tile_pool(name="const", bufs=1))
        work = ctx.enter_context(tc.tile_pool(name="work", bufs=3))
        prodp = ctx.enter_context(tc.tile_pool(name="prod", bufs=4))
        psum_acc = ctx.enter_context(tc.tile_pool(name="psum_acc", bufs=1, space="PSUM"))
        psum_scr = ctx.enter_context(tc.tile_pool(name="psum_scr", bufs=2, space="PSUM"))

        ident = const.tile([P, P], FDT, tag="ident")
        masks.make_identity(nc, ident[:])

        ones_col = const.tile([P, 1], FDT, tag="ones_col")
        nc.vector.memset(ones_col[:], 1.0)

        # ---------- preamble: dec[b,k] = sum_h oh[b,h] W[k,h] + bf[k] ----------
        oh_row = const.tile([1, BL * HIDDEN], FDT, tag="oh_row")
        nc.sync.dma_start(oh_row[:], oh.ap().rearrange("b h -> (b h)").rearrange("(o f) -> o f", o=1))

        w_sb = const.tile([P, KC * HIDDEN], FDT, tag="w_sb")  # col = kc*512 + h
        nc.sync.dma_start(w_sb[:], wf.ap().rearrange("(kc p) h -> p kc h", p=P))

        bf_row = const.tile([1, HIDDEN], FDT, tag="bf_row")
        nc.sync.dma_start(bf_row[:], bf.ap().rearrange("(o h) -> o h", o=1))

        v_row = const.tile([1, HIDDEN], FDT, tag="v_row")
        nc.sync.dma_start(v_row[:], va.ap().rearrange("(o h) -> o h", o=1))

        oh_bc = const.tile([P, BL * HIDDEN], FDT, tag="oh_bc")  # col = b*512 + h
        nc.gpsimd.partition_broadcast(oh_bc[:], oh_row[:])
        v_bc = const.tile([P, HIDDEN], FDT, tag="v_bc")
        nc.gpsimd.partition_broadcast(v_bc[:], v_row[:])

        # decT[p=k_lo, b*KC+kc] = dec[b, kc*128+p] (pre-bias)
        decT = const.tile([P, BL * KC], FDT, tag="decT")
        for b in range(BL):
            for kc in range(KC):
                scr = prodp.tile([P, HIDDEN], FDT, tag="scr")
                c = b * KC + kc
                nc.vector.scalar_tensor_tensor(
                    out=scr[:],
                    in0=w_sb[:, kc * HIDDEN : (kc + 1) * HIDDEN],
                    scalar=1.0,
                    in1=oh_bc[:, b * HIDDEN : (b + 1) * HIDDEN],
                    op0=F32.mult,
                    op1=F32.mult,
                    accum_out=decT[:, c : c + 1],
                )

        # transpose [128, BL*KC] -> [BL*KC, 128]; row r=(b,kc) holds dec[b, kc*128:...]
        pdt = psum_scr.tile([BL * KC, P], FDT, tag="pscr", name="pdt")
        nc.tensor.transpose(pdt[:], decT[:], ident[:])
        dec8 = const.tile([BL * KC, P], FDT, tag="dec8")
        nc.scalar.copy(dec8[:], pdt[:])

        # gather rows into one partition: dec_row[0, b*512 + kc*128 + j]
        dec_row = const.tile([1, BL * HIDDEN], FDT, tag="dec_row")
        nc.sync.dma_start(dec_row[:].rearrange("o (r j) -> o r j", r=BL * KC), dec8[:])
        for b in range(BL):  # + b_feat
            nc.vector.tensor_add(
                dec_row[:, b * HIDDEN : (b + 1) * HIDDEN],
                dec_row[:, b * HIDDEN : (b + 1) * HIDDEN],
                bf_row[:],
            )

        dec_bc = const.tile([P, BL * HIDDEN], FDT, tag="dec_bc")
        nc.gpsimd.partition_broadcast(dec_bc[:], dec_row[:])

        # ---------- mask / temporal_scores_sum ----------
        # staged [c, (b, p)] (512B-contiguous lines), PE-transposed to [p, (c, b)]
        ts_all = const.tile([P, NCHUNK * BL], FDT, tag="ts_all")
        mk_all = const.tile([P, NCHUNK * BL], FDT, tag="mk_all")
        for name, dram, dst in (("ts", tss, ts_all), ("mk", msk, mk_all)):
            stage = const.tile([NCHUNK, BL * P], FDT, tag=f"{name}_stage", name=f"{name}_stage")
            nc.sync.dma_start(
                stage[:].rearrange("c (b p) -> c b p", b=BL),
                dram.ap().rearrange("b (c p) -> c b p", p=P),
            )
            for b in range(BL):
                pt = psum_scr.tile([P, NCHUNK], FDT, tag="pscr", name=f"p{name}{b}")
                nc.tensor.transpose(
                    pt[:], stage[:, b * P : (b + 1) * P], ident[:NCHUNK, :NCHUNK]
                )
                nc.scalar.copy(
                    dst[:].rearrange("p (c b) -> p c b", b=BL)[:, :, b : b + 1], pt[:]
                )

        minv = const.tile([P, NCHUNK * BL], FDT, tag="minv")
        nc.vector.reciprocal(minv[:], ts_all[:])
        nc.vector.tensor_mul(minv[:], minv[:], mk_all[:])

        scores = const.tile([P, NCHUNK * BL], FDT, tag="scores")
        exp_all = const.tile([P, NCHUNK * BL], FDT, tag="exp_all")
        att1 = const.tile([P, NCHUNK * BL], FDT, tag="att1")

        pctx = [
            psum_acc.tile([1, HIDDEN], FDT, tag=f"pctx{b}", name=f"pctx{b}")
            for b in range(BL)
        ]

        # ---------- main streaming loop ----------
        for c in range(NCHUNK):
            eft = work.tile([P, BL * HIDDEN], FDT, tag="eft")
            nc.sync.dma_start(eft[:], ef.ap()[c * P : (c + 1) * P])
            enct = work.tile([P, BL * HIDDEN], FDT, tag="enct")
            nc.sync.dma_start(enct[:], enc.ap()[c * P : (c + 1) * P])

            # ef += dec (one half per engine to keep DVE under the DMA floor)
            nc.gpsimd.tensor_add(
                eft[:, :HIDDEN], eft[:, :HIDDEN], dec_bc[:, :HIDDEN]
            )
            nc.vector.tensor_add(
                eft[:, HIDDEN:], eft[:, HIDDEN:], dec_bc[:, HIDDEN:]
            )
            nc.scalar.activation(eft[:], eft[:], AFT.Tanh)

            for b in range(BL):
                col = c * BL + b
                scr = prodp.tile([P, HIDDEN], FDT, tag="scr")
                nc.vector.scalar_tensor_tensor(
                    out=scr[:],
                    in0=eft[:, b * HIDDEN : (b + 1) * HIDDEN],
                    scalar=1.0,
                    in1=v_bc[:],
                    op0=F32.mult,
                    op1=F32.mult,
                    accum_out=scores[:, col : col + 1],
                )

            cols = slice(c * BL, (c + 1) * BL)
            nc.scalar.activation(exp_all[:, cols], scores[:, cols], AFT.Exp)
            nc.vector.tensor_mul(att1[:, cols], exp_all[:, cols], minv[:, cols])

            for b in range(BL):
                col = c * BL + b
                nc.tensor.matmul(
                    pctx[b][:],
                    att1[:, col : col + 1],
                    enct[:, b * HIDDEN : (b + 1) * HIDDEN],
                    start=(c == 0),
                    stop=(c == NCHUNK - 1),
                )

        # ---------- epilogue ----------
        # denom[b] = sum over (p, c) of att1
        part = const.tile([P, BL], FDT, tag="part")
        nc.vector.reduce_sum(
            part[:], att1[:].rearrange("p (c b) -> p b c", b=BL), axis=mybir.AxisListType.X
        )
        pden = psum_scr.tile([1, BL], FDT, tag="pscr", name="pden")
        nc.tensor.matmul(pden[:], ones_col[:], part[:], start=True, stop=True)
        rd_row = const.tile([1, BL], FDT, tag="rd_row")
        nc.vector.reciprocal(rd_row[:], pden[:])
        rd_bc = const.tile([P, BL], FDT, tag="rd_bc")
        nc.gpsimd.partition_broadcast(rd_bc[:], rd_row[:])

        att_f = const.tile([P, NCHUNK * BL], FDT, tag="att_f")
        for b in range(BL):
            nc.vector.tensor_scalar_mul(
                att_f[:].rearrange("p (c b) -> p c b", b=BL)[:, :, b : b + 1],
                att1[:].rearrange("p (c b) -> p c b", b=BL)[:, :, b : b + 1],
                rd_bc[:, b : b + 1],
            )

        nss = const.tile([P, NCHUNK * BL], FDT, tag="nss")
        nc.vector.tensor_add(nss[:], exp_all[:], ts_all[:])

        # stores: transpose [p, c] -> [c, p] so DRAM writes are contiguous
        for name, src, dram in (("att", att_f, att_o), ("nss", nss, nss_o)):
            stage = const.tile([NCHUNK, BL * P], FDT, tag=f"{name}_ostage", name=f"{name}_ostage")
            for b in range(BL):
                po = psum_scr.tile([NCHUNK, P], FDT, tag="pscr", name=f"po{name}{b}")
                nc.tensor.transpose(
                    po[:],
                    src[:].rearrange("p (c b) -> p c b", b=BL)[:, :, b : b + 1],
                    ident[:],
                )
                nc.scalar.copy(stage[:, b * P : (b + 1) * P], po[:])
            nc.sync.dma_start(
                dram.ap().rearrange("b (c p) -> c b p", p=P),
                stage[:].rearrange("c (b p) -> c b p", b=BL),
            )

        ctx_sb = const.tile([1, BL * HIDDEN], FDT, tag="ctx_sb")
        for b in range(BL):
            nc.vector.tensor_scalar_mul(
                ctx_sb[:, b * HIDDEN : (b + 1) * HIDDEN], pctx[b][:], rd_row[:, b : b + 1]
            )
        nc.sync.dma_start(ctx_o.ap(), ctx_sb[:])

    nc.compile()
    return nc


_NC = None


def _get_nc():
    global _NC
    if _NC is None:
        _NC = build_nc()
    return _NC


def _shard(inputs, i):
    b0, b1 = i * BL, (i + 1) * BL
    f32 = lambda a: np.ascontiguousarray(np.asarray(a), dtype=np.float32)
    return {
        "outputs_hidden": f32(inputs["outputs_hidden"][0, b0:b1, :]),
        "encoder_out": f32(inputs["encoder_out"][:, b0:b1, :]),
        "encoder_features": f32(inputs["encoder_features"][:, b0:b1, :]),
        "encoder_mask": f32(inputs["encoder_mask"][b0:b1, 0, :]),
        "temporal_scores_sum": f32(inputs["temporal_scores_sum"][b0:b1, 0, :]),
        "W_feat": f32(inputs["W_feat"]),
        "b_feat": f32(inputs["b_feat"]),
        "v_attn": f32(inputs["v_attn"]),
    }


def kernel(**inputs):
    nc = _get_nc()
    in_maps = [_shard(inputs, i) for i in range(N_CORES)]
    res = run_bass_kernel_spmd(nc, in_maps, list(range(N_CORES))).results

    context = np.empty((1, BATCH, HIDDEN), np.float32)
    attention = np.empty((BATCH, 1, SEQ), np.float32)
    new_scores_sum = np.empty((BATCH, 1, SEQ), np.float32)
    for i in range(N_CORES):
        b0, b1 = i * BL, (i + 1) * BL
        context[0, b0:b1] = res[i]["context"].reshape(BL, HIDDEN)
        attention[b0:b1, 0] = res[i]["attention"]
        new_scores_sum[b0:b1, 0] = res[i]["new_scores_sum"]
    return context, attention, new_scores_sum
